# revision 15
# baseline (speedup 1.0000x reference)
"""Trainium2 Bass kernel for GQA multi-head attention with RoPE (causal).

Sharding (8 NeuronCores): 2-way data parallel over batch x 4-way sequence
parallel within each batch group.
  - core c: batch b = c//4, group rank j = c%4
  - KV: core computes K/V projections (+RoPE on K) for its contiguous 512-row
    chunk of the sequence, then AllGather over the 4-core group.
  - Q: core owns the strided query rows {j, j+4, j+8, ...} of its batch (512
    rows). Striding makes causal attention work identical on every core, so
    one SPMD program serves all 8 cores; causality enters only through
    host-supplied multiplicative {0,1} mask tables (per-core data).
  - Attention computed in transposed layout (scores^T: kv on partitions) so
    PV needs no transposes. Causal masking is a bf16 multiply AFTER exp
    (exp(s)*m == exp(s+log m) for m in {0,1}).
  - Softmax row sums: bf16 adds on DVE reduce the exp tiles to one
    [128, 256] tile per (head, pair), then a single ones-stationary matmul
    gives the [1, 256] denominators (removes ~170k srow matmul cycles/core
    from the PE). Normalizers broadcast via gpsimd.
  - Per-head loop interleaves Q projection (+RoPE, bf16), attention, and the
    output projection of the previous 8-head group so the tensor engine
    stays dense across phase boundaries (HAM stays un-throttled). Weight
    loads are software-prefetched two heads ahead.
  - Output projection partials accumulate into `out` via DMA-add; host
    scatters rows back into the full (B, S, D) output. No output collective.

All matmuls run in bf16 with fp32 PSUM accumulation.
"""

import os
import sys

sys.path.insert(0, "/opt/trn_rl_repo")
os.environ.setdefault("NEURON_RT_RESET_CORES", "1")

import numpy as np
import ml_dtypes

import concourse.bass as bass  # noqa: F401  (registers engine classes)
import concourse.bacc as bacc
import concourse.mybir as mybir
import concourse.tile as tile
from concourse.bass_utils import run_bass_kernel_spmd

BF16 = ml_dtypes.bfloat16

B, S, D = 2, 2048, 4096
H, KVH, DH = 32, 8, 128
ROPE_BASE = 10000.0
NCORES, TPG = 8, 4          # total cores, cores per batch group
KVC = S // TPG              # 512: kv rows per core
TQ = S // TPG               # 512: query rows per core
KC = D // 128               # 32: contraction chunks of 128
KT = S // 128               # 16: kv tiles per batch
HG = H // 4                 # 8: heads per output-projection group
SCALE = 1.0 / float(np.sqrt(DH))
F32 = mybir.dt.float32
BF = mybir.dt.bfloat16
GROUPS = [[0, 1, 2, 3], [4, 5, 6, 7]]

_NC = None


def _rope_bf(nc, tmp_pool, ps, cos_sb, sin_sb, out_bf):
    """RoPE in [dh, t] layout, bf16 arithmetic:
    out = q*cos + rotate_half(q)*sin with q = bf16(ps)."""
    T = ps.shape[-1]
    qb = tmp_pool.tile([128, T], BF, tag="rope_q")
    nc.vector.tensor_copy(qb[:], ps[:])
    tcos = tmp_pool.tile([128, T], BF, tag="rope_c")
    tsin = tmp_pool.tile([128, T], BF, tag="rope_s")
    nc.vector.tensor_mul(tcos[:], qb[:], cos_sb[:])
    # sin table halves are identical (emb = concat([freqs, freqs])), so the
    # cross-half multiply can read sin at the SAME base partition as qb —
    # required when both DVE inputs are in SBUF.
    nc.vector.tensor_mul(tsin[0:64, :], qb[64:128, :], sin_sb[64:128, :])
    nc.vector.tensor_mul(tsin[64:128, :], qb[0:64, :], sin_sb[0:64, :])
    nc.vector.tensor_sub(out_bf[0:64, :], tcos[0:64, :], tsin[0:64, :])
    nc.vector.tensor_add(out_bf[64:128, :], tcos[64:128, :], tsin[64:128, :])


def _build(sim_single_core=False):
    nd = 1 if sim_single_core else NCORES
    nc = bacc.Bacc("TRN2", target_bir_lowering=False, debug=False, num_devices=nd)

    xq = nc.declare_dram_parameter("xq", [D, TQ], BF, isOutput=False)
    xkv = nc.declare_dram_parameter("xkv", [D, KVC], BF, isOutput=False)
    wq = nc.declare_dram_parameter("wq", [D, D], BF, isOutput=False)
    wk = nc.declare_dram_parameter("wk", [D, KVH * DH], BF, isOutput=False)
    wv = nc.declare_dram_parameter("wv", [D, KVH * DH], BF, isOutput=False)
    wo = nc.declare_dram_parameter("wo", [D, D], BF, isOutput=False)
    cos_q = nc.declare_dram_parameter("cos_q", [DH, TQ], BF, isOutput=False)
    sin_q = nc.declare_dram_parameter("sin_q", [DH, TQ], BF, isOutput=False)
    cos_kv = nc.declare_dram_parameter("cos_kv", [DH, KVC], BF, isOutput=False)
    sin_kv = nc.declare_dram_parameter("sin_kv", [DH, KVC], BF, isOutput=False)
    dmask = nc.declare_dram_parameter("dmask", [128, 8, 256], BF, isOutput=False)
    out = nc.declare_dram_parameter("out", [TQ, D], F32, isOutput=True)

    k_sh = nc.dram_tensor("k_sh", [KVH, DH, KVC], BF)
    v_sh = nc.dram_tensor("v_sh", [KVC, KVH * DH], BF)
    k_g = nc.dram_tensor("k_g", [TPG, KVH, DH, KVC], BF)
    v_g = nc.dram_tensor("v_g", [TPG, KVC, KVH * DH], BF)

    with tile.TileContext(nc) as tc:
        with (
            tc.tile_pool(name="const", bufs=1) as const,
            tc.tile_pool(name="top", bufs=1) as top,
        ):
            ones = const.tile([128, 1], BF)
            nc.vector.memset(ones[:], 1.0)

            # persistent tiles; DMAs issue early and overlap phase 1
            xq_sb = top.tile([128, KC, TQ], BF)
            xq_r = xq.rearrange("(kc p) t -> p kc t", p=128)
            for c4 in range(4):
                nc.sync.dma_start(
                    xq_sb[:, c4 * 8 : (c4 + 1) * 8],
                    xq_r[:, c4 * 8 : (c4 + 1) * 8],
                )
            cosq_sb = top.tile([128, TQ], BF)
            sinq_sb = top.tile([128, TQ], BF)
            nc.sync.dma_start(cosq_sb[:], cos_q[:])
            nc.sync.dma_start(sinq_sb[:], sin_q[:])
            dm_sb = top.tile([128, 8, 256], BF)
            nc.sync.dma_start(dm_sb[:], dmask[:])
            k_sb = top.tile([128, TPG, KVH, KVC], BF)
            v_sb = top.tile([128, KT, KVH * DH], BF)

            # ---- Phase 1: KV projection + RoPE(K) + AllGather ----
            with (
                tc.tile_pool(name="p1x", bufs=1) as p1x,
                tc.tile_pool(name="p1w", bufs=2) as p1w,
                tc.tile_pool(name="p1t", bufs=1) as p1t,
                tc.tile_pool(name="p1o", bufs=3) as p1o,
                tc.tile_pool(name="p1ps", bufs=3, space="PSUM") as p1ps,
            ):
                xkv_sb = p1x.tile([128, KC, KVC], BF)
                xkv_r = xkv.rearrange("(kc p) t -> p kc t", p=128)
                for c4 in range(4):
                    nc.sync.dma_start(
                        xkv_sb[:, c4 * 8 : (c4 + 1) * 8],
                        xkv_r[:, c4 * 8 : (c4 + 1) * 8],
                    )
                coskv_sb = p1x.tile([128, KVC], BF)
                sinkv_sb = p1x.tile([128, KVC], BF)
                nc.sync.dma_start(coskv_sb[:], cos_kv[:])
                nc.sync.dma_start(sinkv_sb[:], sin_kv[:])

                for kp in range(KVH // 2):
                    wk_h = p1w.tile([128, KC, 2 * DH], BF, tag="w")
                    nc.sync.dma_start(
                        wk_h[:],
                        wk[:, kp * 2 * DH : (kp + 1) * 2 * DH].rearrange(
                            "(kc p) c -> p kc c", p=128
                        ),
                    )
                    for hh in range(2):
                        kvh = 2 * kp + hh
                        ps = p1ps.tile([128, KVC], F32, tag="p1ps")
                        for kc in range(KC):
                            nc.tensor.matmul(
                                ps[:],
                                wk_h[:, kc, hh * DH : (hh + 1) * DH],
                                xkv_sb[:, kc],
                                start=(kc == 0), stop=(kc == KC - 1),
                            )
                        k_out = p1o.tile([128, KVC], BF, tag="k_out")
                        _rope_bf(nc, p1t, ps, coskv_sb, sinkv_sb, k_out)
                        nc.sync.dma_start(k_sh[kvh], k_out[:])

                if sim_single_core:
                    for g in range(TPG):
                        nc.sync.dma_start(k_g[g], k_sh[:])
                else:
                    nc.gpsimd.collective_compute(
                        "AllGather", mybir.AluOpType.bypass,
                        replica_groups=GROUPS, ins=[k_sh[:]], outs=[k_g[:]],
                    )

                # V projection, wv streamed in 4 column chunks
                for vc in range(4):
                    wv_c = p1w.tile([128, KC, 256], BF, tag="w")
                    nc.sync.dma_start(
                        wv_c[:],
                        wv[:, vc * 256 : (vc + 1) * 256].rearrange(
                            "(kc p) c -> p kc c", p=128
                        ),
                    )
                    for t4 in range(KVC // 128):
                        ps = p1ps.tile([128, 256], F32, tag="p1ps")
                        for kc in range(KC):
                            nc.tensor.matmul(
                                ps[:],
                                xkv_sb[:, kc, t4 * 128 : (t4 + 1) * 128],
                                wv_c[:, kc],
                                start=(kc == 0), stop=(kc == KC - 1),
                            )
                        v_out = p1o.tile([128, 256], BF, tag="v_out")
                        nc.vector.tensor_copy(v_out[:], ps[:])
                        nc.sync.dma_start(
                            v_sh[t4 * 128 : (t4 + 1) * 128,
                                 vc * 256 : (vc + 1) * 256],
                            v_out[:],
                        )

                if sim_single_core:
                    for g in range(TPG):
                        nc.sync.dma_start(v_g[g], v_sh[:])
                else:
                    nc.gpsimd.collective_compute(
                        "AllGather", mybir.AluOpType.bypass,
                        replica_groups=GROUPS, ins=[v_sh[:]], outs=[v_g[:]],
                    )

            # gathered K/V into SBUF: one 1MB DMA per source core
            for g in range(TPG):
                nc.sync.dma_start(
                    k_sb[:, g], k_g[g].rearrange("kvh p t -> p kvh t")
                )
                nc.sync.dma_start(
                    v_sb[:, 4 * g : 4 * (g + 1)],
                    v_g[g].rearrange("(kt p) c -> p kt c", p=128),
                )

            # ---- Phase 2: per-head Q proj + attention, Wo interleaved ----
            with (
                tc.tile_pool(name="qw", bufs=2) as qwp,
                tc.tile_pool(name="rt", bufs=1) as rtp,
                tc.tile_pool(name="qT", bufs=3) as qTp,
                tc.tile_pool(name="pT", bufs=2) as pTp,
                tc.tile_pool(name="at", bufs=2) as atp,
                tc.tile_pool(name="srt", bufs=2) as srtp,
                tc.tile_pool(name="nrm", bufs=2) as nrm,
                tc.tile_pool(name="wop", bufs=2) as wop,
                tc.tile_pool(name="ob", bufs=2) as obp,
                tc.tile_pool(name="psP", bufs=2, space="PSUM") as psP,
                tc.tile_pool(name="psS", bufs=3, space="PSUM") as psS,
                tc.tile_pool(name="psV", bufs=3, space="PSUM") as psV,
            ):
                qT = {}

                def emit_qproj(h):
                    wq_h = qwp.tile([128, KC, DH], BF, tag="wq",
                                    name=f"wq{h}")
                    nc.sync.dma_start(
                        wq_h[:],
                        wq[:, h * DH : (h + 1) * DH].rearrange(
                            "(kc p) c -> p kc c", p=128
                        ),
                    )
                    ps = psP.tile([128, TQ], F32, tag="proj")
                    for kc in range(KC):
                        nc.tensor.matmul(
                            ps[:],
                            wq_h[:, kc],
                            xq_sb[:, kc],
                            start=(kc == 0), stop=(kc == KC - 1),
                        )
                    qTh = qTp.tile([128, TQ], BF, tag="qT")
                    _rope_bf(nc, rtp, ps, cosq_sb, sinq_sb, qTh)
                    qT[h] = qTh

                def emit_attention(h, attnTg, hh):
                    qTh = qT.pop(h)
                    kvh = h // (H // KVH)
                    for p in range(2):
                        n_kt = 8 * p + 8
                        n_full = n_kt - 4
                        pT = pTp.tile([128, KT, 256], BF, tag="pT")
                        pvt = psV.tile([128, 512], F32, tag="pv")
                        for k2 in range(n_kt // 2):
                            rr = k2 - 4 * p
                            hi = rr >= 2
                            sT = psS.tile([128, 2, 256], F32, tag="sT")
                            for u in range(2):
                                kt = 2 * k2 + u
                                ksl = k_sb[:, kt // 4, kvh,
                                           (kt % 4) * 128 : (kt % 4 + 1) * 128]
                                if hi:
                                    nc.tensor.matmul(
                                        sT[:, u, 0:128], ksl,
                                        qTh[:, p * 256 + 128 : p * 256 + 256],
                                        start=(u == 0), stop=(u == 1),
                                    )
                                else:
                                    nc.tensor.matmul(
                                        sT[:, u, :], ksl,
                                        qTh[:, p * 256 : (p + 1) * 256],
                                        start=(u == 0), stop=(u == 1),
                                    )
                            r = 2 * rr
                            if hi:
                                nc.scalar.activation(
                                    pT[:, 2 * k2 : 2 * k2 + 2, 128:256],
                                    sT[:, :, 0:128],
                                    mybir.ActivationFunctionType.Exp,
                                    scale=SCALE,
                                )
                                nc.vector.tensor_mul(
                                    pT[:, 2 * k2 : 2 * k2 + 2, 128:256],
                                    pT[:, 2 * k2 : 2 * k2 + 2, 128:256],
                                    dm_sb[:, r : r + 2, 128:256],
                                )
                            else:
                                nc.scalar.activation(
                                    pT[:, 2 * k2 : 2 * k2 + 2, :], sT[:],
                                    mybir.ActivationFunctionType.Exp,
                                    scale=SCALE,
                                )
                                if r >= 0:
                                    nc.vector.tensor_mul(
                                        pT[:, 2 * k2 : 2 * k2 + 2, :],
                                        pT[:, 2 * k2 : 2 * k2 + 2, :],
                                        dm_sb[:, r : r + 2, :],
                                    )
                            for u in range(2):
                                kt = 2 * k2 + u
                                if hi:
                                    nc.tensor.matmul(
                                        pvt[:, 128:256],
                                        v_sb[:, kt, kvh * DH : (kvh + 1) * DH],
                                        pT[:, kt, 128:256],
                                        start=False, stop=(kt == n_kt - 1),
                                    )
                                else:
                                    nc.tensor.matmul(
                                        pvt[:, 0:256],
                                        v_sb[:, kt, kvh * DH : (kvh + 1) * DH],
                                        pT[:, kt, :],
                                        start=(kt == 0), stop=(kt == n_kt - 1),
                                    )
                        # softmax denominators: DVE tree + one ones-matmul
                        tsum = srtp.tile([128, 256], BF, tag="tsum")
                        nc.vector.tensor_add(tsum[:], pT[:, 0, :], pT[:, 1, :])
                        for kt in range(2, n_full):
                            nc.vector.tensor_add(tsum[:], tsum[:], pT[:, kt, :])
                        for kt in range(n_full, n_kt):
                            nc.vector.tensor_add(
                                tsum[:, 128:256], tsum[:, 128:256],
                                pT[:, kt, 128:256],
                            )
                        nc.tensor.matmul(
                            pvt[0:1, 256:512], ones[:], tsum[:],
                            start=True, stop=True,
                        )
                        recip = nrm.tile([1, 256], F32, tag="recip")
                        nc.vector.reciprocal(recip[:], pvt[0:1, 256:512])
                        bc = nrm.tile([128, 256], F32, tag="bc")
                        nc.gpsimd.partition_broadcast(bc[:], recip[:])
                        nc.vector.tensor_mul(
                            attnTg[:, hh, p * 256 : (p + 1) * 256],
                            pvt[:, 0:256], bc[:],
                        )

                def emit_wo(g, attnTg):
                    for nn in range(D // 512):
                        wo_g = wop.tile([128, HG, 512], BF, tag="wo")
                        nc.sync.dma_start(
                            wo_g[:],
                            wo[g * HG * 128 : (g + 1) * HG * 128,
                               nn * 512 : (nn + 1) * 512].rearrange(
                                "(h p) c -> p h c", p=128
                            ),
                        )
                        for tq in range(TQ // 128):
                            ps = psP.tile([128, 512], F32, tag="proj")
                            for hh in range(HG):
                                nc.tensor.matmul(
                                    ps[:],
                                    attnTg[:, hh, tq * 128 : (tq + 1) * 128],
                                    wo_g[:, hh],
                                    start=(hh == 0), stop=(hh == HG - 1),
                                )
                            osb = obp.tile([128, 512], F32, tag="osb")
                            oslice = out[tq * 128 : (tq + 1) * 128,
                                         nn * 512 : (nn + 1) * 512]
                            nc.vector.tensor_copy(osb[:], ps[:])
                            if g == 0:
                                nc.sync.dma_start(oslice, osb[:])
                            else:
                                nc.gpsimd.dma_start(
                                    oslice, osb[:],
                                    accum_op=mybir.AluOpType.add,
                                )

                # software-pipelined head loop: Q proj runs 2 heads ahead;
                # each group's Wo is emitted one head into the next group.
                attn_groups = {}
                emit_qproj(0)
                emit_qproj(1)
                for h in range(H):
                    if h + 2 < H:
                        emit_qproj(h + 2)
                    g, hh = divmod(h, HG)
                    if hh == 0:
                        attn_groups[g] = atp.tile([128, HG, TQ], BF,
                                                  tag="attnTg",
                                                  name=f"attnTg{g}")
                    emit_attention(h, attn_groups[g], hh)
                    if h >= HG + 1 and (h - HG - 1) % HG == 0:
                        gdone = (h - HG - 1) // HG
                        emit_wo(gdone, attn_groups.pop(gdone))
                emit_wo(3, attn_groups.pop(3))

    nc.compile()
    return nc


def _get_nc():
    global _NC
    if _NC is None:
        _NC = _build()
    return _NC


def _rope_tables_T(positions):
    """cos/sin tables in [DH, T] layout for given absolute positions."""
    inv_freq = 1.0 / (ROPE_BASE ** (np.arange(0, DH, 2, dtype=np.float64) / DH))
    freqs = inv_freq[:, None] * positions[None, :].astype(np.float64)  # (64, T)
    emb = np.concatenate([freqs, freqs], axis=0)  # (128, T)
    return np.cos(emb).astype(BF16), np.sin(emb).astype(BF16)


def _diag_masks(j):
    """Multiplicative mask table [128 kv, 8 rel-tiles, 256 q], group rank j."""
    i = np.arange(128)
    jj = np.arange(128)
    m = np.zeros((128, 8, 256), dtype=np.float32)
    for r in range(8):
        kvpos = 128 * r + jj[:, None]           # (128, 1)
        lo = kvpos <= 4 * i[None, :] + j        # (128, 128)
        hi = kvpos <= 512 + 4 * i[None, :] + j
        m[:, r, 0:128][lo] = 1.0
        m[:, r, 128:256][hi] = 1.0
    return m.astype(BF16)


def make_in_maps(x, Wq, Wk, Wv, Wo, bo=None):
    wq_bf = Wq.astype(BF16)
    wk_bf = Wk.astype(BF16)
    wv_bf = Wv.astype(BF16)
    wo_bf = Wo.astype(BF16)
    in_maps = []
    for c in range(NCORES):
        b, j = divmod(c, TPG)
        qpos = np.arange(j, S, TPG)
        kvpos = np.arange(j * KVC, (j + 1) * KVC)
        cq, sq = _rope_tables_T(qpos)
        ckv, skv = _rope_tables_T(kvpos)
        in_maps.append({
            "xq": np.ascontiguousarray(x[b, qpos, :].T).astype(BF16),
            "xkv": np.ascontiguousarray(x[b, kvpos, :].T).astype(BF16),
            "wq": wq_bf, "wk": wk_bf, "wv": wv_bf, "wo": wo_bf,
            "cos_q": cq, "sin_q": sq, "cos_kv": ckv, "sin_kv": skv,
            "dmask": _diag_masks(j),
        })
    return in_maps


def assemble_output(results):
    out = np.empty((B, S, D), dtype=np.float32)
    for c in range(NCORES):
        b, j = divmod(c, TPG)
        out[b, j::TPG, :] = results[c]["out"]
    return out


def kernel(x, Wq, Wk, Wv, Wo, bo):
    nc = _get_nc()
    in_maps = make_in_maps(
        np.asarray(x, dtype=np.float32), np.asarray(Wq), np.asarray(Wk),
        np.asarray(Wv), np.asarray(Wo),
    )
    res = run_bass_kernel_spmd(nc, in_maps, list(range(NCORES)))
    full = assemble_output(res.results)
    full += np.asarray(bo, dtype=np.float32)[None, None, :]
    return full


# revision 18
# speedup vs baseline: 1.1020x; 1.1020x over previous
"""Trainium2 Bass kernel for GQA multi-head attention with RoPE (causal).

Sharding (8 NeuronCores): 2-way data parallel over batch x 4-way sequence
parallel within each batch group.
  - core c: batch b = c//4, group rank j = c%4
  - KV: core computes K/V projections (+RoPE on K) for its contiguous 512-row
    chunk of the sequence, then AllGather over the 4-core group.
  - Q: core owns the strided query rows {j, j+4, j+8, ...} of its batch (512
    rows). Striding makes causal attention work identical on every core, so
    one SPMD program serves all 8 cores; causality enters only through
    host-supplied multiplicative {0,1} mask tables (per-core data).
  - Attention computed in transposed layout (scores^T: kv on partitions) so
    PV needs no transposes. Causal masking is a bf16 multiply AFTER exp
    (exp(s)*m == exp(s+log m) for m in {0,1}).
  - Softmax row sums: bf16 adds on DVE reduce the exp tiles to one
    [128, 256] tile per (head, pair), then a single ones-stationary matmul
    gives the [1, 256] denominators (removes ~170k srow matmul cycles/core
    from the PE). Normalizers broadcast via gpsimd.
  - Per-head loop interleaves Q projection (+RoPE, bf16), attention, and the
    output projection of the previous 8-head group so the tensor engine
    stays dense across phase boundaries (HAM stays un-throttled). Weight
    loads are software-prefetched two heads ahead.
  - Output projection partials accumulate into `out` via DMA-add; host
    scatters rows back into the full (B, S, D) output. No output collective.

All matmuls run in bf16 with fp32 PSUM accumulation.
"""

import os
import sys

sys.path.insert(0, "/opt/trn_rl_repo")
os.environ.setdefault("NEURON_RT_RESET_CORES", "1")

import numpy as np
import ml_dtypes

import concourse.bass as bass  # noqa: F401  (registers engine classes)
import concourse.bacc as bacc
import concourse.mybir as mybir
import concourse.tile as tile
from concourse.bass_utils import run_bass_kernel_spmd

BF16 = ml_dtypes.bfloat16

B, S, D = 2, 2048, 4096
H, KVH, DH = 32, 8, 128
ROPE_BASE = 10000.0
NCORES, TPG = 8, 4          # total cores, cores per batch group
KVC = S // TPG              # 512: kv rows per core
TQ = S // TPG               # 512: query rows per core
KC = D // 128               # 32: contraction chunks of 128
KT = S // 128               # 16: kv tiles per batch
HG = H // 4                 # 8: heads per output-projection group
SCALE = 1.0 / float(np.sqrt(DH))
F32 = mybir.dt.float32
BF = mybir.dt.bfloat16
GROUPS = [[0, 1, 2, 3], [4, 5, 6, 7]]

_NC = None


def _rope_bf(nc, tmp_pool, ps, cos_sb, sin_sb, out_bf):
    """RoPE in [dh, t] layout, bf16 arithmetic:
    out = q*cos + rotate_half(q)*sin with q = bf16(ps)."""
    T = ps.shape[-1]
    qb = tmp_pool.tile([128, T], BF, tag="rope_q")
    nc.vector.tensor_copy(qb[:], ps[:])
    tcos = tmp_pool.tile([128, T], BF, tag="rope_c")
    tsin = tmp_pool.tile([128, T], BF, tag="rope_s")
    nc.vector.tensor_mul(tcos[:], qb[:], cos_sb[:])
    # sin table halves are identical (emb = concat([freqs, freqs])), so the
    # cross-half multiply can read sin at the SAME base partition as qb —
    # required when both DVE inputs are in SBUF.
    nc.vector.tensor_mul(tsin[0:64, :], qb[64:128, :], sin_sb[64:128, :])
    nc.vector.tensor_mul(tsin[64:128, :], qb[0:64, :], sin_sb[0:64, :])
    nc.vector.tensor_sub(out_bf[0:64, :], tcos[0:64, :], tsin[0:64, :])
    nc.vector.tensor_add(out_bf[64:128, :], tcos[64:128, :], tsin[64:128, :])


def _build(sim_single_core=False):
    nd = 1 if sim_single_core else NCORES
    nc = bacc.Bacc("TRN2", target_bir_lowering=False, debug=False, num_devices=nd)

    xq = nc.declare_dram_parameter("xq", [D, TQ], BF, isOutput=False)
    xkv = nc.declare_dram_parameter("xkv", [D, KVC], BF, isOutput=False)
    wq = nc.declare_dram_parameter("wq", [D, D], BF, isOutput=False)
    wk = nc.declare_dram_parameter("wk", [D, KVH * DH], BF, isOutput=False)
    wv = nc.declare_dram_parameter("wv", [D, KVH * DH], BF, isOutput=False)
    wo = nc.declare_dram_parameter("wo", [D, D], BF, isOutput=False)
    cos_q = nc.declare_dram_parameter("cos_q", [DH, TQ], BF, isOutput=False)
    sin_q = nc.declare_dram_parameter("sin_q", [DH, TQ], BF, isOutput=False)
    cos_kv = nc.declare_dram_parameter("cos_kv", [DH, KVC], BF, isOutput=False)
    sin_kv = nc.declare_dram_parameter("sin_kv", [DH, KVC], BF, isOutput=False)
    dmask = nc.declare_dram_parameter("dmask", [128, 8, 256], BF, isOutput=False)
    out = nc.declare_dram_parameter("out", [TQ, D], F32, isOutput=True)

    k_sh = nc.dram_tensor("k_sh", [KVH, DH, KVC], BF)
    v_sh = nc.dram_tensor("v_sh", [KVC, KVH * DH], BF)
    k_g = nc.dram_tensor("k_g", [TPG, KVH, DH, KVC], BF)
    v_g = nc.dram_tensor("v_g", [TPG, KVC, KVH * DH], BF)

    with tile.TileContext(nc) as tc:
        with (
            tc.tile_pool(name="const", bufs=1) as const,
            tc.tile_pool(name="top", bufs=1) as top,
        ):
            ones = const.tile([128, 1], BF)
            nc.vector.memset(ones[:], 1.0)

            # persistent tiles; DMAs issue early and overlap phase 1
            xq_sb = top.tile([128, KC, TQ], BF)
            xq_r = xq.rearrange("(kc p) t -> p kc t", p=128)
            for c4 in range(4):
                nc.sync.dma_start(
                    xq_sb[:, c4 * 8 : (c4 + 1) * 8],
                    xq_r[:, c4 * 8 : (c4 + 1) * 8],
                )
            cosq_sb = top.tile([128, TQ], BF)
            sinq_sb = top.tile([128, TQ], BF)
            nc.sync.dma_start(cosq_sb[:], cos_q[:])
            nc.sync.dma_start(sinq_sb[:], sin_q[:])
            dm_sb = top.tile([128, 8, 256], BF)
            nc.sync.dma_start(dm_sb[:], dmask[:])
            k_sb = top.tile([128, TPG, KVH, KVC], BF)
            v_sb = top.tile([128, KT, KVH * DH], BF)

            # ---- Phase 1: KV projection + RoPE(K) + AllGather ----
            with (
                tc.tile_pool(name="p1x", bufs=1) as p1x,
                tc.tile_pool(name="p1w", bufs=2) as p1w,
                tc.tile_pool(name="p1t", bufs=1) as p1t,
                tc.tile_pool(name="p1o", bufs=3) as p1o,
                tc.tile_pool(name="p1ps", bufs=3, space="PSUM") as p1ps,
            ):
                xkv_sb = p1x.tile([128, KC, KVC], BF)
                xkv_r = xkv.rearrange("(kc p) t -> p kc t", p=128)
                for c4 in range(4):
                    nc.sync.dma_start(
                        xkv_sb[:, c4 * 8 : (c4 + 1) * 8],
                        xkv_r[:, c4 * 8 : (c4 + 1) * 8],
                    )
                coskv_sb = p1x.tile([128, KVC], BF)
                sinkv_sb = p1x.tile([128, KVC], BF)
                nc.sync.dma_start(coskv_sb[:], cos_kv[:])
                nc.sync.dma_start(sinkv_sb[:], sin_kv[:])

                for kp in range(KVH // 2):
                    wk_h = p1w.tile([128, KC, 2 * DH], BF, tag="w")
                    nc.sync.dma_start(
                        wk_h[:],
                        wk[:, kp * 2 * DH : (kp + 1) * 2 * DH].rearrange(
                            "(kc p) c -> p kc c", p=128
                        ),
                    )
                    for hh in range(2):
                        kvh = 2 * kp + hh
                        ps = p1ps.tile([128, KVC], F32, tag="p1ps")
                        for kc in range(KC):
                            nc.tensor.matmul(
                                ps[:],
                                wk_h[:, kc, hh * DH : (hh + 1) * DH],
                                xkv_sb[:, kc],
                                start=(kc == 0), stop=(kc == KC - 1),
                            )
                        k_out = p1o.tile([128, KVC], BF, tag="k_out")
                        _rope_bf(nc, p1t, ps, coskv_sb, sinkv_sb, k_out)
                        nc.sync.dma_start(k_sh[kvh], k_out[:])

                if sim_single_core:
                    for g in range(TPG):
                        nc.sync.dma_start(k_g[g], k_sh[:])
                else:
                    nc.gpsimd.collective_compute(
                        "AllGather", mybir.AluOpType.bypass,
                        replica_groups=GROUPS, ins=[k_sh[:]], outs=[k_g[:]],
                    )

                # V projection, wv streamed in 4 column chunks
                for vc in range(4):
                    wv_c = p1w.tile([128, KC, 256], BF, tag="w")
                    nc.sync.dma_start(
                        wv_c[:],
                        wv[:, vc * 256 : (vc + 1) * 256].rearrange(
                            "(kc p) c -> p kc c", p=128
                        ),
                    )
                    for t4 in range(KVC // 128):
                        ps = p1ps.tile([128, 256], F32, tag="p1ps")
                        for kc in range(KC):
                            nc.tensor.matmul(
                                ps[:],
                                xkv_sb[:, kc, t4 * 128 : (t4 + 1) * 128],
                                wv_c[:, kc],
                                start=(kc == 0), stop=(kc == KC - 1),
                            )
                        v_out = p1o.tile([128, 256], BF, tag="v_out")
                        nc.vector.tensor_copy(v_out[:], ps[:])
                        nc.sync.dma_start(
                            v_sh[t4 * 128 : (t4 + 1) * 128,
                                 vc * 256 : (vc + 1) * 256],
                            v_out[:],
                        )

                if sim_single_core:
                    for g in range(TPG):
                        nc.sync.dma_start(v_g[g], v_sh[:])
                else:
                    nc.gpsimd.collective_compute(
                        "AllGather", mybir.AluOpType.bypass,
                        replica_groups=GROUPS, ins=[v_sh[:]], outs=[v_g[:]],
                    )

            # gathered K/V into SBUF: one 1MB DMA per source core
            for g in range(TPG):
                nc.sync.dma_start(
                    k_sb[:, g], k_g[g].rearrange("kvh p t -> p kvh t")
                )
                nc.sync.dma_start(
                    v_sb[:, 4 * g : 4 * (g + 1)],
                    v_g[g].rearrange("(kt p) c -> p kt c", p=128),
                )

            # ---- Phase 2: per-head Q proj + attention, Wo interleaved ----
            with (
                tc.tile_pool(name="qw", bufs=2) as qwp,
                tc.tile_pool(name="rt", bufs=1) as rtp,
                tc.tile_pool(name="qT", bufs=8) as qTp,
                tc.tile_pool(name="pT", bufs=4) as pTp,
                tc.tile_pool(name="at", bufs=2) as atp,
                tc.tile_pool(name="srt", bufs=2) as srtp,
                tc.tile_pool(name="nrm", bufs=2) as nrm,
                tc.tile_pool(name="wop", bufs=2) as wop,
                tc.tile_pool(name="ob", bufs=2) as obp,
                tc.tile_pool(name="psP", bufs=2, space="PSUM") as psP,
                tc.tile_pool(name="psS", bufs=3, space="PSUM") as psS,
                tc.tile_pool(name="psV", bufs=3, space="PSUM") as psV,
            ):
                qT = {}

                def emit_qproj(h):
                    wq_h = qwp.tile([128, KC, DH], BF, tag="wq",
                                    name=f"wq{h}")
                    nc.sync.dma_start(
                        wq_h[:],
                        wq[:, h * DH : (h + 1) * DH].rearrange(
                            "(kc p) c -> p kc c", p=128
                        ),
                    )
                    ps = psP.tile([128, TQ], F32, tag="proj")
                    for kc in range(KC):
                        nc.tensor.matmul(
                            ps[:],
                            wq_h[:, kc],
                            xq_sb[:, kc],
                            start=(kc == 0), stop=(kc == KC - 1),
                        )
                    qTh = qTp.tile([128, TQ], BF, tag="qT")
                    _rope_bf(nc, rtp, ps, cosq_sb, sinq_sb, qTh)
                    qT[h] = qTh

                def emit_attention(h, attnTg, hh):
                    qTh = qT.pop(h)
                    kvh = h // (H // KVH)
                    for p in range(2):
                        n_kt = 8 * p + 8
                        pvt = psV.tile([128, 512], F32, tag="pv")
                        tsum = srtp.tile([128, 256], BF, tag="tsum")
                        for k2 in range(n_kt // 2):
                            rr = k2 - 4 * p
                            hi = rr >= 2
                            sT = psS.tile([128, 2, 256], F32, tag="sT")
                            for u in range(2):
                                kt = 2 * k2 + u
                                ksl = k_sb[:, kt // 4, kvh,
                                           (kt % 4) * 128 : (kt % 4 + 1) * 128]
                                if hi:
                                    nc.tensor.matmul(
                                        sT[:, u, 0:128], ksl,
                                        qTh[:, p * 256 + 128 : p * 256 + 256],
                                        start=(u == 0), stop=(u == 1),
                                    )
                                else:
                                    nc.tensor.matmul(
                                        sT[:, u, :], ksl,
                                        qTh[:, p * 256 : (p + 1) * 256],
                                        start=(u == 0), stop=(u == 1),
                                    )
                            r = 2 * rr
                            pT2 = pTp.tile([128, 2, 256], BF, tag="pT2")
                            if hi:
                                nc.scalar.activation(
                                    pT2[:, :, 128:256],
                                    sT[:, :, 0:128],
                                    mybir.ActivationFunctionType.Exp,
                                    scale=SCALE,
                                )
                                nc.vector.tensor_mul(
                                    pT2[:, :, 128:256],
                                    pT2[:, :, 128:256],
                                    dm_sb[:, r : r + 2, 128:256],
                                )
                            else:
                                nc.scalar.activation(
                                    pT2[:], sT[:],
                                    mybir.ActivationFunctionType.Exp,
                                    scale=SCALE,
                                )
                                if r >= 0:
                                    nc.vector.tensor_mul(
                                        pT2[:], pT2[:],
                                        dm_sb[:, r : r + 2, :],
                                    )
                            # running softmax-denominator sum (bf16, DVE)
                            if k2 == 0:
                                nc.vector.tensor_add(
                                    tsum[:], pT2[:, 0, :], pT2[:, 1, :])
                            elif hi:
                                for u in range(2):
                                    nc.vector.tensor_add(
                                        tsum[:, 128:256], tsum[:, 128:256],
                                        pT2[:, u, 128:256],
                                    )
                            else:
                                for u in range(2):
                                    nc.vector.tensor_add(
                                        tsum[:], tsum[:], pT2[:, u, :])
                            for u in range(2):
                                kt = 2 * k2 + u
                                if hi:
                                    nc.tensor.matmul(
                                        pvt[:, 128:256],
                                        v_sb[:, kt, kvh * DH : (kvh + 1) * DH],
                                        pT2[:, u, 128:256],
                                        start=False, stop=(kt == n_kt - 1),
                                    )
                                else:
                                    nc.tensor.matmul(
                                        pvt[:, 0:256],
                                        v_sb[:, kt, kvh * DH : (kvh + 1) * DH],
                                        pT2[:, u, :],
                                        start=(kt == 0), stop=(kt == n_kt - 1),
                                    )
                        nc.tensor.matmul(
                            pvt[0:1, 256:512], ones[:], tsum[:],
                            start=True, stop=True,
                        )
                        recip = nrm.tile([1, 256], F32, tag="recip")
                        nc.vector.reciprocal(recip[:], pvt[0:1, 256:512])
                        bc = nrm.tile([128, 256], F32, tag="bc")
                        nc.gpsimd.partition_broadcast(bc[:], recip[:])
                        nc.vector.tensor_mul(
                            attnTg[:, hh, p * 256 : (p + 1) * 256],
                            pvt[:, 0:256], bc[:],
                        )

                def emit_wo(g, attnTg):
                    for nn in range(D // 512):
                        wo_g = wop.tile([128, HG, 512], BF, tag="wo")
                        nc.sync.dma_start(
                            wo_g[:],
                            wo[g * HG * 128 : (g + 1) * HG * 128,
                               nn * 512 : (nn + 1) * 512].rearrange(
                                "(h p) c -> p h c", p=128
                            ),
                        )
                        for tq in range(TQ // 128):
                            ps = psP.tile([128, 512], F32, tag="proj")
                            for hh in range(HG):
                                nc.tensor.matmul(
                                    ps[:],
                                    attnTg[:, hh, tq * 128 : (tq + 1) * 128],
                                    wo_g[:, hh],
                                    start=(hh == 0), stop=(hh == HG - 1),
                                )
                            osb = obp.tile([128, 512], F32, tag="osb")
                            oslice = out[tq * 128 : (tq + 1) * 128,
                                         nn * 512 : (nn + 1) * 512]
                            nc.vector.tensor_copy(osb[:], ps[:])
                            if g == 0:
                                nc.sync.dma_start(oslice, osb[:])
                            else:
                                nc.gpsimd.dma_start(
                                    oslice, osb[:],
                                    accum_op=mybir.AluOpType.add,
                                )

                # software-pipelined head loop: Q proj runs 2 heads ahead;
                # each group's Wo is emitted one head into the next group.
                attn_groups = {}
                QDEPTH = 8
                for h0 in range(QDEPTH):
                    emit_qproj(h0)
                for h in range(H):
                    if h + QDEPTH < H:
                        emit_qproj(h + QDEPTH)
                    g, hh = divmod(h, HG)
                    if hh == 0:
                        attn_groups[g] = atp.tile([128, HG, TQ], BF,
                                                  tag="attnTg",
                                                  name=f"attnTg{g}")
                    emit_attention(h, attn_groups[g], hh)
                    if h >= HG + 1 and (h - HG - 1) % HG == 0:
                        gdone = (h - HG - 1) // HG
                        emit_wo(gdone, attn_groups.pop(gdone))
                emit_wo(3, attn_groups.pop(3))

    nc.compile()
    return nc


def _get_nc():
    global _NC
    if _NC is None:
        _NC = _build()
    return _NC


def _rope_tables_T(positions):
    """cos/sin tables in [DH, T] layout for given absolute positions."""
    inv_freq = 1.0 / (ROPE_BASE ** (np.arange(0, DH, 2, dtype=np.float64) / DH))
    freqs = inv_freq[:, None] * positions[None, :].astype(np.float64)  # (64, T)
    emb = np.concatenate([freqs, freqs], axis=0)  # (128, T)
    return np.cos(emb).astype(BF16), np.sin(emb).astype(BF16)


def _diag_masks(j):
    """Multiplicative mask table [128 kv, 8 rel-tiles, 256 q], group rank j."""
    i = np.arange(128)
    jj = np.arange(128)
    m = np.zeros((128, 8, 256), dtype=np.float32)
    for r in range(8):
        kvpos = 128 * r + jj[:, None]           # (128, 1)
        lo = kvpos <= 4 * i[None, :] + j        # (128, 128)
        hi = kvpos <= 512 + 4 * i[None, :] + j
        m[:, r, 0:128][lo] = 1.0
        m[:, r, 128:256][hi] = 1.0
    return m.astype(BF16)


def make_in_maps(x, Wq, Wk, Wv, Wo, bo=None):
    wq_bf = Wq.astype(BF16)
    wk_bf = Wk.astype(BF16)
    wv_bf = Wv.astype(BF16)
    wo_bf = Wo.astype(BF16)
    in_maps = []
    for c in range(NCORES):
        b, j = divmod(c, TPG)
        qpos = np.arange(j, S, TPG)
        kvpos = np.arange(j * KVC, (j + 1) * KVC)
        cq, sq = _rope_tables_T(qpos)
        ckv, skv = _rope_tables_T(kvpos)
        in_maps.append({
            "xq": np.ascontiguousarray(x[b, qpos, :].T).astype(BF16),
            "xkv": np.ascontiguousarray(x[b, kvpos, :].T).astype(BF16),
            "wq": wq_bf, "wk": wk_bf, "wv": wv_bf, "wo": wo_bf,
            "cos_q": cq, "sin_q": sq, "cos_kv": ckv, "sin_kv": skv,
            "dmask": _diag_masks(j),
        })
    return in_maps


def assemble_output(results):
    out = np.empty((B, S, D), dtype=np.float32)
    for c in range(NCORES):
        b, j = divmod(c, TPG)
        out[b, j::TPG, :] = results[c]["out"]
    return out


def kernel(x, Wq, Wk, Wv, Wo, bo):
    nc = _get_nc()
    in_maps = make_in_maps(
        np.asarray(x, dtype=np.float32), np.asarray(Wq), np.asarray(Wk),
        np.asarray(Wv), np.asarray(Wo),
    )
    res = run_bass_kernel_spmd(nc, in_maps, list(range(NCORES)))
    full = assemble_output(res.results)
    full += np.asarray(bo, dtype=np.float32)[None, None, :]
    return full
